# Initial kernel scaffold
#
"""Trainium2 Bass kernel for BiDecoder edge dot products.

out[e] = dot(ufeat[src[e]], ifeat[dst[e]])   for E=300000 edges, D=256.

Strategy (8 NeuronCores, SPMD):
  - Shard edges across the 8 cores (37500 each); replicate ufeat (fp16),
    and give each core a host-compacted ifeat table holding only its
    distinct dst rows (~26.4k < 32767, one int16 gather base). fp16
    halves gather bytes vs f32 at 10-bit-mantissa accuracy (rel err 3e-4).
  - Edges are dst-sorted, so the compacted dst row ids form a dense
    non-decreasing sequence: ~84% of edges pair with a neighbour whose
    row is exactly +1. Paired edges share one 1KB gather descriptor
    (elem_size=512 elems, elem_step=256 over an overlapping table AP),
    cutting both DMA packets (~52ns/1KB vs 2x36ns/512B) and SWDGE
    descriptor generation (~8.3ns/idx-entry per queue, 4 queues).
  - Pair chunks (1024 pairs = 2048 edges: 1 paired hv gather + 2 hu
    gathers) and single chunks (1024 edges: 1+1) run through separate
    slot pools; 4 SWDGE queues rotate per call.
  - DVE: tensor_tensor(mult) for the product (2-byte packed dtype -> 2x
    DVE mode) + tensor_reduce(axis=X) row sums; one final DMA writes the
    dots. Host reorders to original edge order.
"""

import sys

for _p in ("/opt/trn_rl_repo",):
    if _p not in sys.path:
        sys.path.append(_p)

import numpy as np

F16 = np.float16

P = 128
D = 256
E = 300000
NCORES = 8
ECORE = E // NCORES
N_GENE = 20000
N_CELL = 50000
CHUNK_E = 1024           # idx entries per dma_gather call (65 ring descs)
COLS = CHUNK_E // 16     # idx columns per call in the wrapped layout
NSLOT_P = 4              # pair-chunk buffer slots
NSLOT_S = 4              # single-chunk buffer slots

_PROGRAM_CACHE: dict = {}


def _cdiv(a, b):
    return -(-a // b)


def _wrap_idx(idx_i16: np.ndarray, ncall: int) -> np.ndarray:
    """[ncall*CHUNK_E] int16 -> [128, ncall*COLS] dma_gather idx layout."""
    w = idx_i16.reshape(ncall, COLS, 16).transpose(2, 0, 1).reshape(16, ncall * COLS)
    return np.ascontiguousarray(np.tile(w, (8, 1)))


def _build_program(npc: int, nsc: int, vcap: int, n_gene: int = N_GENE):
    import concourse.bacc as bacc
    import concourse.bass as bassmod
    import concourse.mybir as mybir
    from concourse.library_config import mlp

    ycols = npc * 16 + max(1, nsc) * 8
    nsc_t = max(1, nsc)

    nc = bacc.Bacc("TRN2", debug=False, num_swdge_queues=4,
                   dynamic_dma_scratch_size=65536)
    ufeat = nc.dram_tensor("ufeat", [n_gene, D], mybir.dt.float16, kind="ExternalInput")
    # one pad row so the overlapping [256-stride, 512-wide] pair AP stays in bounds
    vtab = nc.dram_tensor("vtab", [vcap + 1, D], mybir.dt.float16, kind="ExternalInput")
    sidxp = nc.dram_tensor("sidxp", [P, npc * 2 * COLS], mybir.dt.int16, kind="ExternalInput")
    didxp = nc.dram_tensor("didxp", [P, npc * COLS], mybir.dt.int16, kind="ExternalInput")
    sidxs = nc.dram_tensor("sidxs", [P, nsc_t * COLS], mybir.dt.int16, kind="ExternalInput")
    didxs = nc.dram_tensor("didxs", [P, nsc_t * COLS], mybir.dt.int16, kind="ExternalInput")
    y = nc.dram_tensor("y", [P, ycols], mybir.dt.float32, kind="ExternalOutput")

    with (
        nc.sbuf_tensor("hup", [P, NSLOT_P, 2, 8, D], mybir.dt.float16) as hup,
        nc.sbuf_tensor("hvp", [P, NSLOT_P, 16, D], mybir.dt.float16) as hvp,
        nc.sbuf_tensor("hus", [P, NSLOT_S, 8, D], mybir.dt.float16) as hus,
        nc.sbuf_tensor("hvs", [P, NSLOT_S, 8, D], mybir.dt.float16) as hvs,
        nc.sbuf_tensor("sidxp_sb", [P, npc * 2 * COLS], mybir.dt.int16) as sidxp_sb,
        nc.sbuf_tensor("didxp_sb", [P, npc * COLS], mybir.dt.int16) as didxp_sb,
        nc.sbuf_tensor("sidxs_sb", [P, nsc_t * COLS], mybir.dt.int16) as sidxs_sb,
        nc.sbuf_tensor("didxs_sb", [P, nsc_t * COLS], mybir.dt.int16) as didxs_sb,
        nc.sbuf_tensor("osb", [P, ycols], mybir.dt.float32) as osb,
        nc.semaphore("io") as io,
        nc.semaphore("cons") as cons,
        nc.semaphore("io2") as io2,
        nc.Block(no_gpsimd_drain=True) as block,
        __import__("contextlib").ExitStack() as _stk,
    ):
        gp_sem = [_stk.enter_context(nc.semaphore(f"gp{i}")) for i in range(NSLOT_P)]
        gs_sem = [_stk.enter_context(nc.semaphore(f"gs{i}")) for i in range(NSLOT_S)]

        vtab_pair_ap = bassmod.AP(vtab, 0, [[D, vcap], [1, 2 * D]])

        # interleave singles among pairs (one single after every ~3 pairs)
        schedule = []
        si = 0
        for c in range(npc):
            schedule.append(("P", c))
            if nsc and (c + 1) % max(1, round(npc / nsc)) == 0 and si < nsc:
                schedule.append(("S", si))
                si += 1
        while si < nsc:
            schedule.append(("S", si))
            si += 1
        pair_gidx = {}
        single_gidx = {}
        for g, (kind, c) in enumerate(schedule):
            (pair_gidx if kind == "P" else single_gidx)[c] = g

        @block.gpsimd
        def _(gp):
            gp.load_library(mlp)
            gp.wait_ge(io, 32)
            q = 0
            started_s = False
            for g, (kind, c) in enumerate(schedule):
                if kind == "P":
                    s = c % NSLOT_P
                    if c >= NSLOT_P:
                        gp.wait_ge(cons, pair_gidx[c - NSLOT_P] + 1)
                    # paired hv: 1024 idx entries, 1KB elems striding 512B rows
                    gp.dma_gather(
                        hvp[:, s].rearrange("p (a b) k -> p a (b k)", b=2),
                        vtab_pair_ap,
                        didxp_sb[:, c * COLS : (c + 1) * COLS],
                        CHUNK_E, CHUNK_E, 2 * D, elem_step=D,
                        queue_num=q % 4, single_packet=False,
                    ).then_inc(gp_sem[s], 16)
                    for h in range(2):
                        cols = slice((2 * c + h) * COLS, (2 * c + h + 1) * COLS)
                        gp.dma_gather(
                            hup[:, s, h], ufeat[:, :], sidxp_sb[:, cols],
                            CHUNK_E, CHUNK_E, D,
                            queue_num=(q + 1 + h) % 4, single_packet=False,
                        ).then_inc(gp_sem[s], 16)
                    q += 3
                else:
                    if not started_s:
                        gp.wait_ge(io, 64)
                        started_s = True
                    s = c % NSLOT_S
                    if c >= NSLOT_S:
                        gp.wait_ge(cons, single_gidx[c - NSLOT_S] + 1)
                    cols = slice(c * COLS, (c + 1) * COLS)
                    gp.dma_gather(
                        hus[:, s], ufeat[:, :], sidxs_sb[:, cols],
                        CHUNK_E, CHUNK_E, D,
                        queue_num=q % 4, single_packet=False,
                    ).then_inc(gs_sem[s], 16)
                    gp.dma_gather(
                        hvs[:, s], vtab[: vcap + 1, :], didxs_sb[:, cols],
                        CHUNK_E, CHUNK_E, D,
                        queue_num=(q + 1) % 4, single_packet=False,
                    ).then_inc(gs_sem[s], 16)
                    q += 2
            for s in range(NSLOT_P):
                cntp = (npc - s + NSLOT_P - 1) // NSLOT_P
                if cntp > 0:
                    gp.wait_ge(gp_sem[s], 48 * cntp)
            for s in range(NSLOT_S):
                cnts = (nsc - s + NSLOT_S - 1) // NSLOT_S
                if cnts > 0:
                    gp.wait_ge(gs_sem[s], 32 * cnts)

        @block.vector
        def _(v):
            H = D // 2
            for kind, c in schedule:
                if kind == "P":
                    s = c % NSLOT_P
                    k = c // NSLOT_P + 1
                    v.wait_ge(gp_sem[s], 48 * k)
                    pview = hvp[:, s].rearrange("p (a b) k -> p a (b k)", b=2)
                    for h in range(2):
                        v.tensor_tensor(
                            out=pview[:, :, h * D : (h + 1) * D],
                            in0=hup[:, s, h],
                            in1=pview[:, :, h * D : (h + 1) * D],
                            op=mybir.AluOpType.mult,
                        )
                    # fold k/k+128 (2x-mode add) so the 1x-mode reduce sees half
                    v.tensor_tensor(
                        out=hvp[:, s, :, 0:H],
                        in0=hvp[:, s, :, 0:H],
                        in1=hvp[:, s, :, H:D],
                        op=mybir.AluOpType.add,
                    )
                    v.tensor_reduce(
                        out=osb[:, c * 16 : (c + 1) * 16],
                        in_=hvp[:, s, :, 0:H],
                        axis=mybir.AxisListType.X,
                        op=mybir.AluOpType.add,
                    ).then_inc(cons, 1)
                else:
                    s = c % NSLOT_S
                    k = c // NSLOT_S + 1
                    v.wait_ge(gs_sem[s], 32 * k)
                    v.tensor_tensor(
                        out=hvs[:, s],
                        in0=hus[:, s],
                        in1=hvs[:, s],
                        op=mybir.AluOpType.mult,
                    )
                    v.tensor_tensor(
                        out=hvs[:, s, :, 0:H],
                        in0=hvs[:, s, :, 0:H],
                        in1=hvs[:, s, :, H:D],
                        op=mybir.AluOpType.add,
                    )
                    v.tensor_reduce(
                        out=osb[:, npc * 16 + c * 8 : npc * 16 + (c + 1) * 8],
                        in_=hvs[:, s, :, 0:H],
                        axis=mybir.AxisListType.X,
                        op=mybir.AluOpType.add,
                    ).then_inc(cons, 1)

        @block.sync
        def _(sy):
            sy.dma_start(didxp_sb[:], didxp[:]).then_inc(io, 16)
            sy.dma_start(sidxp_sb[:], sidxp[:]).then_inc(io, 16)
            sy.dma_start(sidxs_sb[:], sidxs[:]).then_inc(io, 16)
            sy.dma_start(didxs_sb[:], didxs[:]).then_inc(io, 16)
            sy.wait_ge(cons, npc + nsc)
            sy.dma_start(y[:, :], osb[:, :]).then_inc(io2, 16)
            sy.wait_ge(io2, 16)

    nc.compile()
    return nc


def _pair_split(d_loc):
    """Greedy pairing of dst-sorted edges whose compacted rows are r, r+1.

    Returns (pa, pb, singles): positions with d_loc[pb] == d_loc[pa]+1.
    """
    n = len(d_loc)
    run_starts = np.flatnonzero(np.r_[True, np.diff(d_loc) != 0])
    run_vals = d_loc[run_starts]
    run_lens = np.diff(np.r_[run_starts, n])
    pa_idx, pb_idx = [], []
    taken_first = np.zeros(len(run_vals), bool)
    for r in range(len(run_vals) - 1):
        if run_vals[r + 1] == run_vals[r] + 1:
            if not (run_lens[r] == 1 and taken_first[r]):
                pa_idx.append(run_starts[r] + run_lens[r] - 1)
                pb_idx.append(run_starts[r + 1])
                taken_first[r + 1] = True
    pa_idx = np.array(pa_idx, np.int64)
    pb_idx = np.array(pb_idx, np.int64)
    used = np.zeros(n, bool)
    if len(pa_idx):
        used[pa_idx] = True
        used[pb_idx] = True
    return pa_idx, pb_idx, np.flatnonzero(~used)


def _prep_core(s_j, d_loc, ids_j, pa, pb, singles, npc, nsc):
    """Build wrapped idx tensors + (eid, ycol, ypart) mapping for one core."""
    np_pad = npc * CHUNK_E
    ns_pad = max(1, nsc) * CHUNK_E
    pu = np.zeros((2, np_pad), np.int16)
    pv = np.zeros(np_pad, np.int16)
    pe = np.full((2, np_pad), -1, np.int64)
    npair = len(pa)
    pu[0, :npair] = s_j[pa].astype(np.int16)
    pu[1, :npair] = s_j[pb].astype(np.int16)
    pv[:npair] = d_loc[pa].astype(np.int16)
    pe[0, :npair] = ids_j[pa]
    pe[1, :npair] = ids_j[pb]
    # sort pairs within each chunk by first-edge src for hu locality
    for c in range(npc):
        sl = slice(c * CHUNK_E, (c + 1) * CHUNK_E)
        perm = np.argsort(pu[0, sl], kind="stable")
        pu[:, sl] = pu[:, sl][:, perm]
        pv[sl] = pv[sl][perm]
        pe[:, sl] = pe[:, sl][:, perm]
    su = np.zeros(ns_pad, np.int16)
    sv = np.zeros(ns_pad, np.int16)
    se = np.full(ns_pad, -1, np.int64)
    nsing = len(singles)
    su[:nsing] = s_j[singles].astype(np.int16)
    sv[:nsing] = d_loc[singles].astype(np.int16)
    se[:nsing] = ids_j[singles]
    for c in range(nsc):
        sl = slice(c * CHUNK_E, (c + 1) * CHUNK_E)
        perm = np.argsort(su[sl], kind="stable")
        su[sl] = su[sl][perm]
        sv[sl] = sv[sl][perm]
        se[sl] = se[sl][perm]
    # hu pair calls: chunk c -> call A then call B, each CHUNK_E entries
    sidxp = _wrap_idx(
        pu.T.reshape(npc, CHUNK_E, 2).transpose(0, 2, 1).reshape(-1), npc * 2)
    didxp = _wrap_idx(pv, npc)
    sidxs = _wrap_idx(su, max(1, nsc))
    didxs = _wrap_idx(sv, max(1, nsc))
    # y mapping: pair i (chunk c, block b=(i - c*1024)//128, part p=i%128),
    # half h -> y[p, c*16 + 2*b + h]; single i -> y[p, npc*16 + c*8 + b]
    eids, cols, parts = [], [], []
    for c in range(npc):
        idx = np.arange(c * CHUNK_E, (c + 1) * CHUNK_E)
        b = (idx - c * CHUNK_E) // 128
        for h in range(2):
            eids.append(pe[h, idx])
            cols.append(c * 16 + 2 * b + h)
            parts.append(idx % 128)
    for c in range(nsc):
        idx = np.arange(c * CHUNK_E, (c + 1) * CHUNK_E)
        b = (idx - c * CHUNK_E) // 128
        eids.append(se[idx])
        cols.append(npc * 16 + c * 8 + b)
        parts.append(idx % 128)
    return (sidxp, didxp, sidxs, didxs,
            np.concatenate(eids), np.concatenate(cols), np.concatenate(parts))


def kernel(ufeat, ifeat, src, dst):
    from concourse.bass_utils import run_bass_kernel_spmd

    ufeat_h = np.ascontiguousarray(np.asarray(ufeat, dtype=np.float32)).astype(F16)
    ifeat_h = np.ascontiguousarray(np.asarray(ifeat, dtype=np.float32)).astype(F16)
    src_f = np.asarray(src).ravel().astype(np.int64)
    dst_f = np.asarray(dst).ravel().astype(np.int64)
    assert src_f.shape == (E,) and dst_f.shape == (E,)

    cores = []
    for j in range(NCORES):
        lo, hi = j * ECORE, (j + 1) * ECORE
        d_j = dst_f[lo:hi]
        order = np.argsort(d_j, kind="stable")
        uniq, d_loc = np.unique(d_j[order], return_inverse=True)
        s_j = src_f[lo:hi][order]
        ids_j = np.arange(lo, hi)[order]
        pa, pb, singles = _pair_split(d_loc)
        cores.append((s_j, d_loc, uniq, ids_j, pa, pb, singles))

    vcap = max(len(u) for (_, _, u, _, _, _, _) in cores)
    npc = max(_cdiv(len(pa), CHUNK_E) for (_, _, _, _, pa, _, _) in cores)
    nsc = max(_cdiv(len(sg), CHUNK_E) for (_, _, _, _, _, _, sg) in cores)

    key = (npc, nsc, vcap)
    if key not in _PROGRAM_CACHE:
        _PROGRAM_CACHE[key] = _build_program(npc, nsc, vcap)
    nc = _PROGRAM_CACHE[key]

    in_maps = []
    maps = []
    for j in range(NCORES):
        s_j, d_loc, uniq, ids_j, pa, pb, singles = cores[j]
        vtab = np.zeros((vcap + 1, D), F16)
        vtab[: len(uniq)] = ifeat_h[uniq]
        sidxp, didxp, sidxs, didxs, eid, ycol, ypart = _prep_core(
            s_j, d_loc, ids_j, pa, pb, singles, npc, nsc)
        in_maps.append({"ufeat": ufeat_h, "vtab": vtab, "sidxp": sidxp,
                        "didxp": didxp, "sidxs": sidxs, "didxs": didxs})
        maps.append((eid, ycol, ypart))

    res = run_bass_kernel_spmd(nc, in_maps, core_ids=list(range(NCORES)))

    out = np.empty((E, 1), np.float32)
    for j in range(NCORES):
        yj = np.asarray(res.results[j]["y"])   # [128, ycols]
        eid, ycol, ypart = maps[j]
        m = eid >= 0
        out[eid[m], 0] = yj[ypart[m], ycol[m]]
    return out



# revision 1
# speedup vs baseline: 1.0044x; 1.0044x over previous
"""Trainium2 Bass kernel for BiDecoder edge dot products.

out[e] = dot(ufeat[src[e]], ifeat[dst[e]])   for E=300000 edges, D=256.

Strategy (8 NeuronCores, SPMD):
  - Shard edges across the 8 cores (37500 each); replicate ufeat (fp16),
    and give each core a host-compacted ifeat table holding only its
    distinct dst rows (~26.4k < 32767, one int16 gather base). fp16
    halves gather bytes vs f32 at 10-bit-mantissa accuracy (rel err 3e-4).
  - Edges are dst-sorted, so the compacted dst row ids form a dense
    non-decreasing sequence: ~84% of edges pair with a neighbour whose
    row is exactly +1. Paired edges share one 1KB gather descriptor
    (elem_size=512 elems, elem_step=256 over an overlapping table AP),
    cutting both DMA packets (~52ns/1KB vs 2x36ns/512B) and SWDGE
    descriptor generation (~8.3ns/idx-entry per queue, 4 queues).
  - Pair chunks (1024 pairs = 2048 edges: 1 paired hv gather + 2 hu
    gathers) and single chunks (1024 edges: 1+1) run through separate
    slot pools; 4 SWDGE queues rotate per call.
  - DVE: tensor_tensor(mult) for the product (2-byte packed dtype -> 2x
    DVE mode) + tensor_reduce(axis=X) row sums; one final DMA writes the
    dots. Host reorders to original edge order.
"""

import sys

for _p in ("/opt/trn_rl_repo",):
    if _p not in sys.path:
        sys.path.append(_p)

import numpy as np

F16 = np.float16

P = 128
D = 256
E = 300000
NCORES = 8
ECORE = E // NCORES
N_GENE = 20000
N_CELL = 50000
CHUNK_E = 1024           # idx entries per dma_gather call (65 ring descs)
COLS = CHUNK_E // 16     # idx columns per call in the wrapped layout
NSLOT_P = 4              # pair-chunk buffer slots
NSLOT_S = 4              # single-chunk buffer slots

_PROGRAM_CACHE: dict = {}


def _cdiv(a, b):
    return -(-a // b)


def _wrap_idx(idx_i16: np.ndarray, ncall: int) -> np.ndarray:
    """[ncall*CHUNK_E] int16 -> [128, ncall*COLS] dma_gather idx layout."""
    w = idx_i16.reshape(ncall, COLS, 16).transpose(2, 0, 1).reshape(16, ncall * COLS)
    return np.ascontiguousarray(np.tile(w, (8, 1)))


def _build_program(npc: int, nsc: int, vcap: int, n_gene: int = N_GENE):
    import concourse.bacc as bacc
    import concourse.bass as bassmod
    import concourse.mybir as mybir
    from concourse.library_config import mlp

    ycols = npc * 16 + max(1, nsc) * 8
    nsc_t = max(1, nsc)

    nc = bacc.Bacc("TRN2", debug=False, num_swdge_queues=4,
                   dynamic_dma_scratch_size=65536)
    ufeat = nc.dram_tensor("ufeat", [n_gene, D], mybir.dt.float16, kind="ExternalInput")
    # one pad row so the overlapping [256-stride, 512-wide] pair AP stays in bounds
    vtab = nc.dram_tensor("vtab", [vcap + 1, D], mybir.dt.float16, kind="ExternalInput")
    sidxp = nc.dram_tensor("sidxp", [P, npc * 2 * COLS], mybir.dt.int16, kind="ExternalInput")
    didxp = nc.dram_tensor("didxp", [P, npc * COLS], mybir.dt.int16, kind="ExternalInput")
    sidxs = nc.dram_tensor("sidxs", [P, nsc_t * COLS], mybir.dt.int16, kind="ExternalInput")
    didxs = nc.dram_tensor("didxs", [P, nsc_t * COLS], mybir.dt.int16, kind="ExternalInput")
    y = nc.dram_tensor("y", [P, ycols], mybir.dt.float32, kind="ExternalOutput")

    with (
        nc.sbuf_tensor("hup", [P, NSLOT_P, 2, 8, D], mybir.dt.float16) as hup,
        nc.sbuf_tensor("hvp", [P, NSLOT_P, 16, D], mybir.dt.float16) as hvp,
        nc.sbuf_tensor("hus", [P, NSLOT_S, 8, D], mybir.dt.float16) as hus,
        nc.sbuf_tensor("hvs", [P, NSLOT_S, 8, D], mybir.dt.float16) as hvs,
        nc.sbuf_tensor("sidxp_sb", [P, npc * 2 * COLS], mybir.dt.int16) as sidxp_sb,
        nc.sbuf_tensor("didxp_sb", [P, npc * COLS], mybir.dt.int16) as didxp_sb,
        nc.sbuf_tensor("sidxs_sb", [P, nsc_t * COLS], mybir.dt.int16) as sidxs_sb,
        nc.sbuf_tensor("didxs_sb", [P, nsc_t * COLS], mybir.dt.int16) as didxs_sb,
        nc.sbuf_tensor("osb", [P, ycols], mybir.dt.float32) as osb,
        nc.semaphore("io") as io,
        nc.semaphore("cons") as cons,
        nc.semaphore("io2") as io2,
        nc.Block(no_gpsimd_drain=True) as block,
        __import__("contextlib").ExitStack() as _stk,
    ):
        gp_sem = [_stk.enter_context(nc.semaphore(f"gp{i}")) for i in range(NSLOT_P)]
        gs_sem = [_stk.enter_context(nc.semaphore(f"gs{i}")) for i in range(NSLOT_S)]

        vtab_pair_ap = bassmod.AP(vtab, 0, [[D, vcap], [1, 2 * D]])

        # interleave singles among pairs (one single after every ~3 pairs)
        schedule = []
        si = 0
        for c in range(npc):
            schedule.append(("P", c))
            if nsc and (c + 1) % max(1, round(npc / nsc)) == 0 and si < nsc:
                schedule.append(("S", si))
                si += 1
        while si < nsc:
            schedule.append(("S", si))
            si += 1
        pair_gidx = {}
        single_gidx = {}
        for g, (kind, c) in enumerate(schedule):
            (pair_gidx if kind == "P" else single_gidx)[c] = g

        @block.gpsimd
        def _(gp):
            gp.load_library(mlp)
            gp.wait_ge(io, 32)
            q = 0
            started_s = False
            for g, (kind, c) in enumerate(schedule):
                if kind == "P":
                    s = c % NSLOT_P
                    if c >= NSLOT_P:
                        gp.wait_ge(cons, pair_gidx[c - NSLOT_P] + 1)
                    # paired hv: 1024 idx entries, 1KB elems striding 512B rows
                    gp.dma_gather(
                        hvp[:, s].rearrange("p (a b) k -> p a (b k)", b=2),
                        vtab_pair_ap,
                        didxp_sb[:, c * COLS : (c + 1) * COLS],
                        CHUNK_E, CHUNK_E, 2 * D, elem_step=D,
                        queue_num=q % 4, single_packet=False,
                    ).then_inc(gp_sem[s], 16)
                    for h in range(2):
                        cols = slice((2 * c + h) * COLS, (2 * c + h + 1) * COLS)
                        gp.dma_gather(
                            hup[:, s, h], ufeat[:, :], sidxp_sb[:, cols],
                            CHUNK_E, CHUNK_E, D,
                            queue_num=(q + 1 + h) % 4, single_packet=False,
                        ).then_inc(gp_sem[s], 16)
                    q += 3
                else:
                    if not started_s:
                        gp.wait_ge(io, 64)
                        started_s = True
                    s = c % NSLOT_S
                    if c >= NSLOT_S:
                        gp.wait_ge(cons, single_gidx[c - NSLOT_S] + 1)
                    cols = slice(c * COLS, (c + 1) * COLS)
                    gp.dma_gather(
                        hus[:, s], ufeat[:, :], sidxs_sb[:, cols],
                        CHUNK_E, CHUNK_E, D,
                        queue_num=q % 4, single_packet=False,
                    ).then_inc(gs_sem[s], 16)
                    gp.dma_gather(
                        hvs[:, s], vtab[: vcap + 1, :], didxs_sb[:, cols],
                        CHUNK_E, CHUNK_E, D,
                        queue_num=(q + 1) % 4, single_packet=False,
                    ).then_inc(gs_sem[s], 16)
                    q += 2
            for s in range(NSLOT_P):
                cntp = (npc - s + NSLOT_P - 1) // NSLOT_P
                if cntp > 0:
                    gp.wait_ge(gp_sem[s], 48 * cntp)
            for s in range(NSLOT_S):
                cnts = (nsc - s + NSLOT_S - 1) // NSLOT_S
                if cnts > 0:
                    gp.wait_ge(gs_sem[s], 32 * cnts)

        @block.vector
        def _(v):
            H = D // 2
            for kind, c in schedule:
                if kind == "P":
                    s = c % NSLOT_P
                    k = c // NSLOT_P + 1
                    v.wait_ge(gp_sem[s], 48 * k)
                    pview = hvp[:, s].rearrange("p (a b) k -> p a (b k)", b=2)
                    for h in range(2):
                        v.tensor_tensor(
                            out=pview[:, :, h * D : (h + 1) * D],
                            in0=hup[:, s, h],
                            in1=pview[:, :, h * D : (h + 1) * D],
                            op=mybir.AluOpType.mult,
                        )
                    # fold k/k+128 (2x-mode add) so the 1x-mode reduce sees half
                    v.tensor_tensor(
                        out=hvp[:, s, :, 0:H],
                        in0=hvp[:, s, :, 0:H],
                        in1=hvp[:, s, :, H:D],
                        op=mybir.AluOpType.add,
                    )
                    v.tensor_reduce(
                        out=osb[:, c * 16 : (c + 1) * 16],
                        in_=hvp[:, s, :, 0:H],
                        axis=mybir.AxisListType.X,
                        op=mybir.AluOpType.add,
                    ).then_inc(cons, 1)
                else:
                    s = c % NSLOT_S
                    k = c // NSLOT_S + 1
                    v.wait_ge(gs_sem[s], 32 * k)
                    v.tensor_tensor(
                        out=hvs[:, s],
                        in0=hus[:, s],
                        in1=hvs[:, s],
                        op=mybir.AluOpType.mult,
                    )
                    v.tensor_tensor(
                        out=hvs[:, s, :, 0:H],
                        in0=hvs[:, s, :, 0:H],
                        in1=hvs[:, s, :, H:D],
                        op=mybir.AluOpType.add,
                    )
                    v.tensor_reduce(
                        out=osb[:, npc * 16 + c * 8 : npc * 16 + (c + 1) * 8],
                        in_=hvs[:, s, :, 0:H],
                        axis=mybir.AxisListType.X,
                        op=mybir.AluOpType.add,
                    ).then_inc(cons, 1)

        @block.sync
        def _(sy):
            sy.dma_start(didxp_sb[:], didxp[:]).then_inc(io, 16)
            sy.dma_start(sidxp_sb[:], sidxp[:]).then_inc(io, 16)
            sy.dma_start(sidxs_sb[:], sidxs[:]).then_inc(io, 16)
            sy.dma_start(didxs_sb[:], didxs[:]).then_inc(io, 16)
            sy.wait_ge(cons, npc + nsc)
            sy.dma_start(y[:, :], osb[:, :]).then_inc(io2, 16)
            sy.wait_ge(io2, 16)

    nc.compile()
    return nc


def _pair_split(d_loc):
    """Greedy pairing of dst-sorted edges whose compacted rows are r, r+1.

    Returns (pa, pb, singles): positions with d_loc[pb] == d_loc[pa]+1.
    """
    n = len(d_loc)
    run_starts = np.flatnonzero(np.r_[True, np.diff(d_loc) != 0])
    run_vals = d_loc[run_starts]
    run_lens = np.diff(np.r_[run_starts, n])
    pa_idx, pb_idx = [], []
    taken_first = np.zeros(len(run_vals), bool)
    for r in range(len(run_vals) - 1):
        if run_vals[r + 1] == run_vals[r] + 1:
            if not (run_lens[r] == 1 and taken_first[r]):
                pa_idx.append(run_starts[r] + run_lens[r] - 1)
                pb_idx.append(run_starts[r + 1])
                taken_first[r + 1] = True
    pa_idx = np.array(pa_idx, np.int64)
    pb_idx = np.array(pb_idx, np.int64)
    used = np.zeros(n, bool)
    if len(pa_idx):
        used[pa_idx] = True
        used[pb_idx] = True
    return pa_idx, pb_idx, np.flatnonzero(~used)


def _prep_core(s_j, d_loc, ids_j, pa, pb, singles, npc, nsc):
    """Build wrapped idx tensors + (eid, ycol, ypart) mapping for one core."""
    np_pad = npc * CHUNK_E
    ns_pad = max(1, nsc) * CHUNK_E
    pu = np.zeros((2, np_pad), np.int16)
    pv = np.zeros(np_pad, np.int16)
    pe = np.full((2, np_pad), -1, np.int64)
    npair = len(pa)
    pu[0, :npair] = s_j[pa].astype(np.int16)
    pu[1, :npair] = s_j[pb].astype(np.int16)
    pv[:npair] = d_loc[pa].astype(np.int16)
    pe[0, :npair] = ids_j[pa]
    pe[1, :npair] = ids_j[pb]
    # sort pairs within each chunk by first-edge src for hu locality
    for c in range(npc):
        sl = slice(c * CHUNK_E, (c + 1) * CHUNK_E)
        perm = np.argsort(pu[0, sl], kind="stable")
        pu[:, sl] = pu[:, sl][:, perm]
        pv[sl] = pv[sl][perm]
        pe[:, sl] = pe[:, sl][:, perm]
    su = np.zeros(ns_pad, np.int16)
    sv = np.zeros(ns_pad, np.int16)
    se = np.full(ns_pad, -1, np.int64)
    nsing = len(singles)
    su[:nsing] = s_j[singles].astype(np.int16)
    sv[:nsing] = d_loc[singles].astype(np.int16)
    se[:nsing] = ids_j[singles]
    for c in range(nsc):
        sl = slice(c * CHUNK_E, (c + 1) * CHUNK_E)
        perm = np.argsort(su[sl], kind="stable")
        su[sl] = su[sl][perm]
        sv[sl] = sv[sl][perm]
        se[sl] = se[sl][perm]
    # hu pair calls: chunk c -> call A then call B, each CHUNK_E entries
    sidxp = _wrap_idx(
        pu.T.reshape(npc, CHUNK_E, 2).transpose(0, 2, 1).reshape(-1), npc * 2)
    didxp = _wrap_idx(pv, npc)
    sidxs = _wrap_idx(su, max(1, nsc))
    didxs = _wrap_idx(sv, max(1, nsc))
    # y mapping: pair i (chunk c, block b=(i - c*1024)//128, part p=i%128),
    # half h -> y[p, c*16 + 2*b + h]; single i -> y[p, npc*16 + c*8 + b]
    eids, cols, parts = [], [], []
    for c in range(npc):
        idx = np.arange(c * CHUNK_E, (c + 1) * CHUNK_E)
        b = (idx - c * CHUNK_E) // 128
        for h in range(2):
            eids.append(pe[h, idx])
            cols.append(c * 16 + 2 * b + h)
            parts.append(idx % 128)
    for c in range(nsc):
        idx = np.arange(c * CHUNK_E, (c + 1) * CHUNK_E)
        b = (idx - c * CHUNK_E) // 128
        eids.append(se[idx])
        cols.append(npc * 16 + c * 8 + b)
        parts.append(idx % 128)
    return (sidxp, didxp, sidxs, didxs,
            np.concatenate(eids), np.concatenate(cols), np.concatenate(parts))


def kernel(ufeat, ifeat, src, dst):
    from concourse.bass_utils import run_bass_kernel_spmd

    ufeat_h = np.ascontiguousarray(np.asarray(ufeat, dtype=np.float32)).astype(F16)
    ifeat_h = np.ascontiguousarray(np.asarray(ifeat, dtype=np.float32)).astype(F16)
    src_f = np.asarray(src).ravel().astype(np.int64)
    dst_f = np.asarray(dst).ravel().astype(np.int64)
    assert src_f.shape == (E,) and dst_f.shape == (E,)

    cores = []
    for j in range(NCORES):
        lo, hi = j * ECORE, (j + 1) * ECORE
        d_j = dst_f[lo:hi]
        order = np.argsort(d_j, kind="stable")
        uniq, d_loc = np.unique(d_j[order], return_inverse=True)
        s_j = src_f[lo:hi][order]
        ids_j = np.arange(lo, hi)[order]
        pa, pb, singles = _pair_split(d_loc)
        cores.append((s_j, d_loc, uniq, ids_j, pa, pb, singles))

    vcap = max(len(u) for (_, _, u, _, _, _, _) in cores)
    npc = max(_cdiv(len(pa), CHUNK_E) for (_, _, _, _, pa, _, _) in cores)
    nsc = max(_cdiv(len(sg), CHUNK_E) for (_, _, _, _, _, _, sg) in cores)

    key = (npc, nsc, vcap)
    if key not in _PROGRAM_CACHE:
        _PROGRAM_CACHE[key] = _build_program(npc, nsc, vcap)
    nc = _PROGRAM_CACHE[key]

    in_maps = []
    maps = []
    for j in range(NCORES):
        s_j, d_loc, uniq, ids_j, pa, pb, singles = cores[j]
        vtab = np.zeros((vcap + 1, D), F16)
        vtab[: len(uniq)] = ifeat_h[uniq]
        sidxp, didxp, sidxs, didxs, eid, ycol, ypart = _prep_core(
            s_j, d_loc, ids_j, pa, pb, singles, npc, nsc)
        in_maps.append({"ufeat": ufeat_h, "vtab": vtab, "sidxp": sidxp,
                        "didxp": didxp, "sidxs": sidxs, "didxs": didxs})
        maps.append((eid, ycol, ypart))

    res = run_bass_kernel_spmd(nc, in_maps, core_ids=list(range(NCORES)))

    out = np.empty((E, 1), np.float32)
    for j in range(NCORES):
        yj = np.asarray(res.results[j]["y"])   # [128, ycols]
        eid, ycol, ypart = maps[j]
        m = eid >= 0
        out[eid[m], 0] = yj[ypart[m], ycol[m]]
    return out

